# revision 7
# baseline (speedup 1.0000x reference)
"""Trainium2 Bass kernel for nn_Decoder (conv-seq2seq decoder).

Strategy: data-parallel over batch, 2 batch elements per NeuronCore (8 cores).
Each core runs the full 6-layer decoder on its 2 batch elements in bf16
matmuls with fp32 accumulation. Weights are host-prepacked into SBUF-friendly
layouts; the residual stream is carried in a per-layer scaled representation
y_l = sqrt(2)^(l+1) * x_l so every layer update is a pure add (all scale
factors are folded into weights or single fused DVE ops).

Self-contained: hardcodes shapes B=16,T=512,S=512,V=10000,E=512,H=1024,L=6,K=3.
"""
import os
import sys
import numpy as np
import ml_dtypes

for _p in ("/opt/trn_rl_repo", "/root/.axon_site/_ro/trn_rl_repo"):
    if os.path.isdir(_p) and _p not in sys.path:
        sys.path.insert(0, _p)

import concourse.bass as bass
import concourse.bacc as bacc
import concourse.mybir as mybir
import concourse.tile as tile
from concourse.bass_utils import run_bass_kernel_spmd
from concourse.bass_interp import get_hw_module
from concourse.masks import make_identity

bf16 = ml_dtypes.bfloat16
f32 = np.float32
BF = mybir.dt.bfloat16
F32 = mybir.dt.float32
I32 = mybir.dt.int32

B, T, S = 16, 512, 512
V, E, H = 10000, 512, 1024
L, K = 6, 3
NCORES = 8
BL = B // NCORES          # batch per core
P = 128
EC, SC, TC, HC, OC = E // P, S // P, T // P, H // P, 2 * H // P   # 4,4,4,8,16
VT = 20                   # v tiles of 512 (V padded 10000 -> 10240)
AluOp = mybir.AluOpType
ACT = mybir.ActivationFunctionType


def _prepack(inputs):
    """Host-side fold + layout prep. Returns dict of per-core-shared arrays
    (weights) and the per-batch arrays to shard."""
    s = f32(np.sqrt(0.5))
    rt2 = f32(np.sqrt(2.0))
    c = np.array([rt2 * rt2 ** l for l in range(L + 1)], f32)

    g = {k: np.asarray(v) for k, v in inputs.items()}
    conv_w = g["conv_w"].astype(f32)
    b1 = g["attn_hid2emb_b"].astype(f32)

    out = {}
    out["tok2"] = (s * g["tok_emb"].astype(f32)).astype(f32)                 # [V, E]
    out["pos2"] = (s * (g["pos_emb"][:T].astype(f32) + b1[None, :])).astype(f32)  # [T, E]
    out["we2h"] = np.ascontiguousarray(
        (2.0 * g["emb2hid_w"].astype(f32)).astype(bf16)
        .reshape(EC, P, H).transpose(1, 0, 2))                                # [P, EC, H]
    out["y0b"] = (rt2 * (g["emb2hid_b"].astype(f32) - b1 @ g["emb2hid_w"].astype(f32))
                  ).astype(f32).reshape(HC, P).T.copy()                       # [P, HC]
    # conv weights: a-half scaled s/2 (GLU computes psum*(tanh+1)), g-half /c_l
    wdev = np.empty((L, 2 * H, H, K), f32)
    wdev[:, :H] = (s / 2.0) * conv_w[:, :H]
    wdev[:, H:] = conv_w[:, H:] / c[:L, None, None, None]
    # [L, OC, P(i_lo), HC(hi), K, P(o_lo)] flattened to [L, OC, P, HC*K*P]
    wc = wdev.reshape(L, OC, P, HC, P, K).transpose(0, 1, 4, 3, 5, 2)
    out["wc"] = np.ascontiguousarray(wc.astype(bf16)).reshape(L, OC, P, HC * K * P)
    out["ba"] = ((c[:L, None] * s / 2.0) * g["conv_b"][:, :H].astype(f32)
                 ).reshape(L, HC, P).transpose(0, 2, 1).copy()                # [L, P, HC]
    out["bg"] = (0.5 * g["conv_b"][:, H:].astype(f32)
                 ).reshape(L, HC, P).transpose(0, 2, 1).copy()                # [L, P, HC]
    out["w1"] = np.ascontiguousarray(
        g["attn_hid2emb_w"].astype(f32).astype(bf16).reshape(HC, P, E).transpose(1, 0, 2))
    out["w2s"] = np.ascontiguousarray(
        (s * g["attn_emb2hid_w"].astype(f32)).astype(bf16).reshape(EC, P, H).transpose(1, 0, 2))
    out["b2s"] = (s * g["attn_emb2hid_b"].astype(f32)).reshape(HC, P).T.copy()  # [P, HC]
    out["whid"] = np.ascontiguousarray(
        (g["hid2emb_w"].astype(f32) / c[L]).astype(bf16).reshape(HC, P, E).transpose(1, 0, 2))
    out["hidb"] = g["hid2emb_b"].astype(f32).reshape(EC, P).T.copy()          # [P, EC]
    wfc = np.zeros((E, VT * 512), f32)
    wfc[:, :V] = g["fc_out_w"].astype(f32)
    out["wfc"] = np.ascontiguousarray(
        wfc.astype(bf16).reshape(EC, P, VT, 512).transpose(1, 2, 0, 3)
    ).reshape(P, VT, EC * 512)                                                # [P, VT, EC*512]
    out["fcb"] = g["fc_out_b"].astype(f32)                                    # host-applied if nonzero
    # per-batch tensors
    out["tgt32"] = g["tgt"].astype(np.int32)                                  # [B, T]
    out["encT"] = np.ascontiguousarray(
        g["encoder_conved"].astype(f32).astype(bf16)
        .reshape(B, S, EC, P).transpose(0, 3, 2, 1))                          # [B, P, EC, S]
    out["encC"] = np.ascontiguousarray(
        g["encoder_combined"].astype(f32).astype(bf16)
        .reshape(B, SC, P, E).transpose(0, 2, 1, 3))                          # [B, P, SC, E]
    out["c"] = c
    return out


def _build_program(pp):
    """Emit the per-core Bass/Tile program. Returns nc."""
    s = f32(np.sqrt(0.5))
    c = pp["c"]
    has_ba = bool(np.any(pp["ba"]))
    has_bg = bool(np.any(pp["bg"]))
    has_y0b = bool(np.any(pp["y0b"]))
    has_b2s = bool(np.any(pp["b2s"]))
    has_hidb = bool(np.any(pp["hidb"]))

    nc = bacc.Bacc("TRN2", target_bir_lowering=False, debug=False,
                   num_devices=NCORES)

    # ---- DRAM I/O ----
    d_tgt = nc.dram_tensor("tgt32", [BL * T], I32, kind="ExternalInput")
    d_tok = nc.dram_tensor("tok2", [V, E], F32, kind="ExternalInput")
    d_pos = nc.dram_tensor("pos2", [T, E], F32, kind="ExternalInput")
    d_we2h = nc.dram_tensor("we2h", [P, EC, H], BF, kind="ExternalInput")
    d_wc = nc.dram_tensor("wc", [L, OC, P, HC * K * P], BF, kind="ExternalInput")
    d_w1 = nc.dram_tensor("w1", [P, HC, E], BF, kind="ExternalInput")
    d_w2s = nc.dram_tensor("w2s", [P, EC, H], BF, kind="ExternalInput")
    d_whid = nc.dram_tensor("whid", [P, HC, E], BF, kind="ExternalInput")
    d_wfc = nc.dram_tensor("wfc", [P, VT, EC * 512], BF, kind="ExternalInput")
    d_encT = nc.dram_tensor("encT", [BL, P, EC, S], BF, kind="ExternalInput")
    d_encC = nc.dram_tensor("encC", [BL, P, SC, E], BF, kind="ExternalInput")
    d_y0b = nc.dram_tensor("y0b", [P, HC], F32, kind="ExternalInput") if has_y0b else None
    d_ba = nc.dram_tensor("ba", [L, P, HC], F32, kind="ExternalInput") if has_ba else None
    d_bg = nc.dram_tensor("bg", [L, P, HC], F32, kind="ExternalInput") if has_bg else None
    d_b2s = nc.dram_tensor("b2s", [P, HC], F32, kind="ExternalInput") if has_b2s else None
    d_hidb = nc.dram_tensor("hidb", [P, EC], F32, kind="ExternalInput") if has_hidb else None
    d_out = nc.dram_tensor("logits", [BL, T, V], F32, kind="ExternalOutput")
    d_attn = nc.dram_tensor("attno", [BL, T, S], F32, kind="ExternalOutput")

    with tile.TileContext(nc) as tc:
        with (
            tc.tile_pool(name="const", bufs=1) as const,
            tc.tile_pool(name="resid", bufs=1) as resid,
            tc.tile_pool(name="wres", bufs=1) as wres,
            tc.tile_pool(name="wstream", bufs=3) as wstream,
            tc.tile_pool(name="work", bufs=2) as work,
            tc.tile_pool(name="actbuf", bufs=2) as actbuf,
            tc.tile_pool(name="outbuf", bufs=2) as outbuf,
            tc.tile_pool(name="cpsum", bufs=4, space="PSUM") as cpsum,
            tc.tile_pool(name="apsum", bufs=3, space="PSUM") as apsum,
            tc.tile_pool(name="tpsum", bufs=1, space="PSUM") as tpsum,
        ):
            # ---- constants / residents ----
            ident = const.tile([P, P], BF)
            make_identity(nc, ident[:])
            ones = const.tile([P, P], BF)
            nc.any.memset(ones[:], 1.0)

            y = resid.tile([P, HC, BL, T], F32, name="y")           # residual, fp32
            xpb = resid.tile([P, HC, BL, T + K - 1], BF, name="xpb")  # conv input, padded
            embT = resid.tile([P, EC, BL, T], BF, name="embT")
            cgb = resid.tile([P, HC, BL, T], BF, name="cgb")
            attnT = resid.tile([P, SC, BL, T], BF, name="attnT")
            comb = resid.tile([P, EC, BL, T], BF, name="comb")
            attT = comb   # alias: attendedT overwrites combined (disjoint live ranges)
            femb = embT   # alias: final emb overwrites embedded (last read is layer L-1)
            encT_sb = resid.tile([P, EC, BL, S], BF, name="encT_sb")
            encC_sb = resid.tile([P, SC, BL, E], BF, name="encC_sb")
            rbc = resid.tile([P, BL, T], F32, name="rbc")           # 1/Z broadcast

            w1_sb = wres.tile([P, HC, E], BF, name="w1_sb")
            w2_sb = wres.tile([P, EC, H], BF, name="w2_sb")
            whid_sb = wres.tile([P, HC, E], BF, name="whid_sb")
            we2h_sb = wres.tile([P, EC, H], BF, name="we2h_sb")
            bias_sb = wres.tile([P, 4 + 2 * L, HC], F32, name="bias_sb")
            # bias_sb columns: 0 y0b, 1 b2s, 2 hidb(EC), 3 spare, 4+2l ba_l, 5+2l bg_l

            nc.sync.dma_start(w1_sb[:], d_w1[:])
            nc.sync.dma_start(w2_sb[:], d_w2s[:])
            nc.sync.dma_start(whid_sb[:], d_whid[:])
            nc.sync.dma_start(we2h_sb[:], d_we2h[:])
            nc.sync.dma_start(encT_sb[:, :, 0, :], d_encT[0])
            nc.sync.dma_start(encT_sb[:, :, 1, :], d_encT[1])
            nc.sync.dma_start(encC_sb[:, :, 0, :], d_encC[0])
            nc.sync.dma_start(encC_sb[:, :, 1, :], d_encC[1])
            if has_y0b:
                nc.sync.dma_start(bias_sb[:, 0, :], d_y0b[:])
            if has_b2s:
                nc.sync.dma_start(bias_sb[:, 1, :], d_b2s[:])
            if has_hidb:
                nc.sync.dma_start(bias_sb[:, 2, :EC], d_hidb[:])
            if has_ba:
                for l in range(L):
                    nc.sync.dma_start(bias_sb[:, 4 + 2 * l, :], d_ba[l])
            if has_bg:
                for l in range(L):
                    nc.sync.dma_start(bias_sb[:, 5 + 2 * l, :], d_bg[l])

            # ---- embedding gather + transpose to [E, T] ----
            for b in range(BL):
                for t4 in range(TC):
                    idx = work.tile([P, 1], I32, tag="idx")
                    nc.sync.dma_start(idx[:], d_tgt[b * T + t4 * P:b * T + (t4 + 1) * P, None])
                    gath = work.tile([P, E], F32, tag="gath")
                    nc.gpsimd.indirect_dma_start(
                        out=gath[:], out_offset=None, in_=d_tok[:, :],
                        in_offset=bass.IndirectOffsetOnAxis(ap=idx[:, :1], axis=0))
                    post = work.tile([P, E], F32, tag="post")
                    nc.sync.dma_start(post[:], d_pos[t4 * P:(t4 + 1) * P, :])
                    epre = work.tile([P, E], BF, tag="epre")
                    nc.vector.tensor_add(out=epre[:], in0=gath[:], in1=post[:])
                    for ec in range(EC):
                        pt = tpsum.tile([P, P], BF, tag="tp")
                        nc.tensor.transpose(pt[:], epre[:, ec * P:(ec + 1) * P], ident[:])
                        nc.scalar.copy(out=embT[:, ec, b, t4 * P:(t4 + 1) * P], in_=pt[:])

            # ---- y0 = 2*emb2hid^T @ embT (+bias) ----
            for b in range(BL):
                for ho in range(HC):
                    ps = cpsum.tile([P, T], F32, tag="cps")
                    for ec in range(EC):
                        nc.tensor.matmul(ps[:], we2h_sb[:, ec, ho * P:(ho + 1) * P],
                                         embT[:, ec, b, :],
                                         start=(ec == 0), stop=(ec == EC - 1))
                    if has_y0b:
                        nc.scalar.activation(y[:, ho, b, :], ps[:], ACT.Identity,
                                             bias=bias_sb[:, 0, ho:ho + 1])
                    else:
                        nc.scalar.copy(out=y[:, ho, b, :], in_=ps[:])
                nc.scalar.copy(out=xpb[:, :, b, K - 1:], in_=y[:, :, b, :])

            # ---- layers ----
            for l in range(L):
                cl = float(c[l])
                for b in range(BL):
                    nc.any.memset(xpb[:, :, b, 0:K - 1], cl)

                # conv + GLU: pairs (a: ho, g: ho+HC)
                for hp in range(HC):
                    wa = wstream.tile([P, HC * K * P], BF, tag="wconv")
                    nc.sync.dma_start(wa[:], d_wc[l, hp])
                    wg = wstream.tile([P, HC * K * P], BF, tag="wconv")
                    nc.sync.dma_start(wg[:], d_wc[l, hp + HC])
                    wa3 = wa.rearrange("p (hi k o) -> p hi k o", hi=HC, k=K)
                    wg3 = wg.rearrange("p (hi k o) -> p hi k o", hi=HC, k=K)
                    for b in range(BL):
                        pa = cpsum.tile([P, T], F32, tag="cps")
                        pg = cpsum.tile([P, T], F32, tag="cps")
                        for hi in range(HC):
                            for k in range(K):
                                st = (hi == 0 and k == 0)
                                sp = (hi == HC - 1 and k == K - 1)
                                nc.tensor.matmul(pa[:], wa3[:, hi, k, :],
                                                 xpb[:, hi, b, k:k + T], start=st, stop=sp)
                        for hi in range(HC):
                            for k in range(K):
                                st = (hi == 0 and k == 0)
                                sp = (hi == HC - 1 and k == K - 1)
                                nc.tensor.matmul(pg[:], wg3[:, hi, k, :],
                                                 xpb[:, hi, b, k:k + T], start=st, stop=sp)
                        th = actbuf.tile([P, T], F32, tag="tanh")
                        if has_bg:
                            nc.scalar.activation(th[:], pg[:], ACT.Tanh, scale=0.5,
                                                 bias=bias_sb[:, 5 + 2 * l, hp:hp + 1])
                        else:
                            nc.scalar.activation(th[:], pg[:], ACT.Tanh, scale=0.5)
                        if has_ba:
                            pa_b = actbuf.tile([P, T], F32, tag="pab")
                            nc.scalar.activation(pa_b[:], pa[:], ACT.Identity,
                                                 bias=bias_sb[:, 4 + 2 * l, hp:hp + 1])
                            a_src = pa_b
                        else:
                            a_src = pa
                        # cg = (tanh + 1) * a   (bf16 out)
                        nc.vector.scalar_tensor_tensor(
                            out=cgb[:, hp, b, :], in0=th[:], scalar=1.0,
                            in1=a_src[:], op0=AluOp.add, op1=AluOp.mult)

                for b in range(BL):
                    # conved_emb -> combined
                    for ec in range(EC):
                        pce = apsum.tile([P, T], F32, tag="aps")
                        for hc_ in range(HC):
                            nc.tensor.matmul(pce[:], w1_sb[:, hc_, ec * P:(ec + 1) * P],
                                             cgb[:, hc_, b, :],
                                             start=(hc_ == 0), stop=(hc_ == HC - 1))
                        nc.vector.scalar_tensor_tensor(
                            out=comb[:, ec, b, :], in0=pce[:], scalar=1.0 / cl,
                            in1=embT[:, ec, b, :], op0=AluOp.mult, op1=AluOp.add)
                    # energyT + exp
                    for sc in range(SC):
                        pen = apsum.tile([P, T], F32, tag="aps")
                        for ec in range(EC):
                            nc.tensor.matmul(pen[:], encT_sb[:, ec, b, sc * P:(sc + 1) * P],
                                             comb[:, ec, b, :],
                                             start=(ec == 0), stop=(ec == EC - 1))
                        nc.scalar.activation(attnT[:, sc, b, :], pen[:], ACT.Exp)
                    # Z broadcast via all-ones matmul, then 1/Z
                    pz = apsum.tile([P, T], F32, tag="aps")
                    for sc in range(SC):
                        nc.tensor.matmul(pz[:], ones[:], attnT[:, sc, b, :],
                                         start=(sc == 0), stop=(sc == SC - 1))
                    nc.vector.reciprocal_approx_fast(out=rbc[:, b, :], in_=pz[:])
                    for sc in range(SC):
                        nc.vector.tensor_mul(out=attnT[:, sc, b, :],
                                             in0=attnT[:, sc, b, :], in1=rbc[:, b, :])
                    # attendedT
                    for ec in range(EC):
                        pat = apsum.tile([P, T], F32, tag="aps")
                        for sc in range(SC):
                            nc.tensor.matmul(pat[:], encC_sb[:, sc, b, ec * P:(ec + 1) * P],
                                             attnT[:, sc, b, :],
                                             start=(sc == 0), stop=(sc == SC - 1))
                        nc.scalar.copy(out=attT[:, ec, b, :], in_=pat[:])
                    # att2 + residual update
                    for ho in range(HC):
                        p2 = apsum.tile([P, T], F32, tag="aps")
                        for ec in range(EC):
                            nc.tensor.matmul(p2[:], w2_sb[:, ec, ho * P:(ho + 1) * P],
                                             attT[:, ec, b, :],
                                             start=(ec == 0), stop=(ec == EC - 1))
                        if has_b2s:
                            p2b = actbuf.tile([P, T], F32, tag="p2b")
                            nc.scalar.activation(p2b[:], p2[:], ACT.Identity,
                                                 bias=bias_sb[:, 1, ho:ho + 1])
                            p2_src = p2b
                        else:
                            p2_src = p2
                        t1 = actbuf.tile([P, T], F32, tag="t1")
                        nc.vector.scalar_tensor_tensor(
                            out=t1[:], in0=p2_src[:], scalar=cl,
                            in1=cgb[:, ho, b, :], op0=AluOp.mult, op1=AluOp.add)
                        nc.vector.tensor_add(out=y[:, ho, b, :], in0=y[:, ho, b, :],
                                             in1=t1[:])
                    # refresh bf16 conv input for next layer / final matmul
                    nc.scalar.copy(out=xpb[:, :, b, K - 1:], in_=y[:, :, b, :])
                    # attention output (last layer): transpose to [T, S] and store
                    if l == L - 1:
                        for t4 in range(TC):
                            ao = outbuf.tile([P, S], F32, tag="ao")
                            for sc in range(SC):
                                pt = tpsum.tile([P, P], BF, tag="tp")
                                nc.tensor.transpose(
                                    pt[:], attnT[:, sc, b, t4 * P:(t4 + 1) * P], ident[:])
                                nc.scalar.copy(out=ao[:, sc * P:(sc + 1) * P], in_=pt[:])
                            nc.sync.dma_start(d_attn[b, t4 * P:(t4 + 1) * P, :], ao[:])

            # ---- final projection: femb = whid^T @ y_L (+hidb) ----
            for b in range(BL):
                for ec in range(EC):
                    pf = cpsum.tile([P, T], F32, tag="cps")
                    for hc_ in range(HC):
                        nc.tensor.matmul(pf[:], whid_sb[:, hc_, ec * P:(ec + 1) * P],
                                         xpb[:, hc_, b, K - 1:],
                                         start=(hc_ == 0), stop=(hc_ == HC - 1))
                    if has_hidb:
                        nc.scalar.activation(femb[:, ec, b, :], pf[:], ACT.Identity,
                                             bias=bias_sb[:, 2, ec:ec + 1])
                    else:
                        nc.scalar.copy(out=femb[:, ec, b, :], in_=pf[:])

            # ---- fc_out: logits[t, v] ----
            for vt in range(VT):
                wf = wstream.tile([P, EC * 512], BF, tag="wfc")
                nc.sync.dma_start(wf[:], d_wfc[:, vt, :])
                wf3 = wf.rearrange("p (e n) -> p e n", e=EC)
                vw = min(512, V - vt * 512)
                for b in range(BL):
                    for t4 in range(TC):
                        pl = cpsum.tile([P, 512], F32, tag="cps")
                        for ec in range(EC):
                            nc.tensor.matmul(pl[:], femb[:, ec, b, t4 * P:(t4 + 1) * P],
                                             wf3[:, ec, :],
                                             start=(ec == 0), stop=(ec == EC - 1))
                        ot = outbuf.tile([P, 512], F32, tag="ot")
                        nc.scalar.copy(out=ot[:, :vw], in_=pl[:, :vw])
                        nc.sync.dma_start(
                            d_out[b, t4 * P:(t4 + 1) * P, vt * 512:vt * 512 + vw],
                            ot[:, :vw])
    return nc


_CACHE = {}


def _run(inputs, trace=False):
    pp = _prepack(inputs)
    if "nc" not in _CACHE:
        nc = _build_program(pp)
        nc.compile()
        nc.m = get_hw_module(nc.m)
        _CACHE["nc"] = nc
    nc = _CACHE["nc"]

    shared_keys = ["tok2", "pos2", "we2h", "wc", "w1", "w2s", "whid", "wfc"]
    in_maps = []
    for core in range(NCORES):
        m = {k: pp[k] for k in shared_keys}
        bsl = slice(core * BL, (core + 1) * BL)
        m["tgt32"] = np.ascontiguousarray(pp["tgt32"][bsl].reshape(-1))
        m["encT"] = np.ascontiguousarray(pp["encT"][bsl])
        m["encC"] = np.ascontiguousarray(pp["encC"][bsl])
        if np.any(pp["y0b"]):
            m["y0b"] = pp["y0b"]
        if np.any(pp["ba"]):
            m["ba"] = pp["ba"]
        if np.any(pp["bg"]):
            m["bg"] = pp["bg"]
        if np.any(pp["b2s"]):
            m["b2s"] = pp["b2s"]
        if np.any(pp["hidb"]):
            m["hidb"] = pp["hidb"]
        in_maps.append(m)

    res = run_bass_kernel_spmd(nc, in_maps, core_ids=list(range(NCORES)),
                               trace=trace)
    out = np.concatenate([r["logits"] for r in res.results], axis=0)
    attn = np.concatenate([r["attno"] for r in res.results], axis=0)
    if np.any(pp["fcb"]):
        out = out + pp["fcb"][None, None, :]
    return (out, attn), res


def kernel(**inputs):
    (out, attn), _ = _run(inputs, trace=False)
    return out, attn


# revision 8
# speedup vs baseline: 1.0415x; 1.0415x over previous
"""Trainium2 Bass kernel for nn_Decoder (conv-seq2seq decoder).

Strategy: data-parallel over batch, 2 batch elements per NeuronCore (8 cores).
Each core runs the full 6-layer decoder on its 2 batch elements in bf16
matmuls with fp32 accumulation. Weights are host-prepacked into SBUF-friendly
layouts; the residual stream is carried in a per-layer scaled representation
y_l = sqrt(2)^(l+1) * x_l so every layer update is a pure add (all scale
factors are folded into weights or single fused DVE ops).

Self-contained: hardcodes shapes B=16,T=512,S=512,V=10000,E=512,H=1024,L=6,K=3.
"""
import os
import sys
import numpy as np
import ml_dtypes

for _p in ("/opt/trn_rl_repo", "/root/.axon_site/_ro/trn_rl_repo"):
    if os.path.isdir(_p) and _p not in sys.path:
        sys.path.insert(0, _p)

import concourse.bass as bass
import concourse.bacc as bacc
import concourse.mybir as mybir
import concourse.tile as tile
from concourse.bass_utils import run_bass_kernel_spmd
from concourse.bass_interp import get_hw_module
from concourse.masks import make_identity

bf16 = ml_dtypes.bfloat16
f32 = np.float32
BF = mybir.dt.bfloat16
F32 = mybir.dt.float32
I32 = mybir.dt.int32

B, T, S = 16, 512, 512
V, E, H = 10000, 512, 1024
L, K = 6, 3
NCORES = 8
BL = B // NCORES          # batch per core
P = 128
EC, SC, TC, HC, OC = E // P, S // P, T // P, H // P, 2 * H // P   # 4,4,4,8,16
VT = 20                   # v tiles of 512 (V padded 10000 -> 10240)
AluOp = mybir.AluOpType
ACT = mybir.ActivationFunctionType


def _prepack(inputs):
    """Host-side fold + layout prep. Returns dict of per-core-shared arrays
    (weights) and the per-batch arrays to shard."""
    s = f32(np.sqrt(0.5))
    rt2 = f32(np.sqrt(2.0))
    c = np.array([rt2 * rt2 ** l for l in range(L + 1)], f32)

    g = {k: np.asarray(v) for k, v in inputs.items()}
    conv_w = g["conv_w"].astype(f32)
    b1 = g["attn_hid2emb_b"].astype(f32)

    out = {}
    out["tok2"] = (s * g["tok_emb"].astype(f32)).astype(f32)                 # [V, E]
    out["pos2"] = (s * (g["pos_emb"][:T].astype(f32) + b1[None, :])).astype(f32)  # [T, E]
    out["we2h"] = np.ascontiguousarray(
        (2.0 * g["emb2hid_w"].astype(f32)).astype(bf16)
        .reshape(EC, P, H).transpose(1, 0, 2))                                # [P, EC, H]
    out["y0b"] = (rt2 * (g["emb2hid_b"].astype(f32) - b1 @ g["emb2hid_w"].astype(f32))
                  ).astype(f32).reshape(HC, P).T.copy()                       # [P, HC]
    # conv weights: a-half scaled s/2 (GLU computes psum*(tanh+1)), g-half /c_l
    wdev = np.empty((L, 2 * H, H, K), f32)
    wdev[:, :H] = (s / 2.0) * conv_w[:, :H]
    wdev[:, H:] = conv_w[:, H:] / c[:L, None, None, None]
    # [L, OC, P(i_lo), HC(hi), K, P(o_lo)] flattened to [L, OC, P, HC*K*P]
    wc = wdev.reshape(L, OC, P, HC, P, K).transpose(0, 1, 4, 3, 5, 2)
    out["wc"] = np.ascontiguousarray(wc.astype(bf16)).reshape(L, OC, P, HC * K * P)
    out["ba"] = ((c[:L, None] * s / 2.0) * g["conv_b"][:, :H].astype(f32)
                 ).reshape(L, HC, P).transpose(0, 2, 1).copy()                # [L, P, HC]
    out["bg"] = (0.5 * g["conv_b"][:, H:].astype(f32)
                 ).reshape(L, HC, P).transpose(0, 2, 1).copy()                # [L, P, HC]
    out["w1"] = np.ascontiguousarray(
        g["attn_hid2emb_w"].astype(f32).astype(bf16).reshape(HC, P, E).transpose(1, 0, 2))
    out["w2s"] = np.ascontiguousarray(
        (s * g["attn_emb2hid_w"].astype(f32)).astype(bf16).reshape(EC, P, H).transpose(1, 0, 2))
    out["b2s"] = (s * g["attn_emb2hid_b"].astype(f32)).reshape(HC, P).T.copy()  # [P, HC]
    out["whid"] = np.ascontiguousarray(
        (g["hid2emb_w"].astype(f32) / c[L]).astype(bf16).reshape(HC, P, E).transpose(1, 0, 2))
    out["hidb"] = g["hid2emb_b"].astype(f32).reshape(EC, P).T.copy()          # [P, EC]
    wfc = np.zeros((E, VT * 512), f32)
    wfc[:, :V] = g["fc_out_w"].astype(f32)
    out["wfc"] = np.ascontiguousarray(
        wfc.astype(bf16).reshape(EC, P, VT, 512).transpose(1, 2, 0, 3)
    ).reshape(P, VT, EC * 512)                                                # [P, VT, EC*512]
    out["fcb"] = g["fc_out_b"].astype(f32)                                    # host-applied if nonzero
    # per-batch tensors
    out["tgt32"] = g["tgt"].astype(np.int32)                                  # [B, T]
    out["encT"] = np.ascontiguousarray(
        g["encoder_conved"].astype(f32).astype(bf16)
        .reshape(B, S, EC, P).transpose(0, 3, 2, 1))                          # [B, P, EC, S]
    out["encC"] = np.ascontiguousarray(
        g["encoder_combined"].astype(f32).astype(bf16)
        .reshape(B, SC, P, E).transpose(0, 2, 1, 3))                          # [B, P, SC, E]
    out["c"] = c
    return out


def _build_program(pp):
    """Emit the per-core Bass/Tile program. Returns nc."""
    s = f32(np.sqrt(0.5))
    c = pp["c"]
    has_ba = bool(np.any(pp["ba"]))
    has_bg = bool(np.any(pp["bg"]))
    has_y0b = bool(np.any(pp["y0b"]))
    has_b2s = bool(np.any(pp["b2s"]))
    has_hidb = bool(np.any(pp["hidb"]))

    nc = bacc.Bacc("TRN2", target_bir_lowering=False, debug=False,
                   num_devices=NCORES)

    # ---- DRAM I/O ----
    d_tgt = nc.dram_tensor("tgt32", [BL * T], I32, kind="ExternalInput")
    d_tok = nc.dram_tensor("tok2", [V, E], F32, kind="ExternalInput")
    d_pos = nc.dram_tensor("pos2", [T, E], F32, kind="ExternalInput")
    d_we2h = nc.dram_tensor("we2h", [P, EC, H], BF, kind="ExternalInput")
    d_wc = nc.dram_tensor("wc", [L, OC, P, HC * K * P], BF, kind="ExternalInput")
    d_w1 = nc.dram_tensor("w1", [P, HC, E], BF, kind="ExternalInput")
    d_w2s = nc.dram_tensor("w2s", [P, EC, H], BF, kind="ExternalInput")
    d_whid = nc.dram_tensor("whid", [P, HC, E], BF, kind="ExternalInput")
    d_wfc = nc.dram_tensor("wfc", [P, VT, EC * 512], BF, kind="ExternalInput")
    d_encT = nc.dram_tensor("encT", [BL, P, EC, S], BF, kind="ExternalInput")
    d_encC = nc.dram_tensor("encC", [BL, P, SC, E], BF, kind="ExternalInput")
    d_y0b = nc.dram_tensor("y0b", [P, HC], F32, kind="ExternalInput") if has_y0b else None
    d_ba = nc.dram_tensor("ba", [L, P, HC], F32, kind="ExternalInput") if has_ba else None
    d_bg = nc.dram_tensor("bg", [L, P, HC], F32, kind="ExternalInput") if has_bg else None
    d_b2s = nc.dram_tensor("b2s", [P, HC], F32, kind="ExternalInput") if has_b2s else None
    d_hidb = nc.dram_tensor("hidb", [P, EC], F32, kind="ExternalInput") if has_hidb else None
    d_out = nc.dram_tensor("logits", [BL, T, V], F32, kind="ExternalOutput")
    d_attn = nc.dram_tensor("attno", [BL, T, S], F32, kind="ExternalOutput")

    with tile.TileContext(nc) as tc:
        with (
            tc.tile_pool(name="const", bufs=1) as const,
            tc.tile_pool(name="resid", bufs=1) as resid,
            tc.tile_pool(name="wres", bufs=1) as wres,
            tc.tile_pool(name="wstream", bufs=3) as wstream,
            tc.tile_pool(name="work", bufs=2) as work,
            tc.tile_pool(name="actbuf", bufs=2) as actbuf,
            tc.tile_pool(name="outbuf", bufs=3) as outbuf,
            tc.tile_pool(name="cpsum", bufs=4, space="PSUM") as cpsum,
            tc.tile_pool(name="apsum", bufs=3, space="PSUM") as apsum,
            tc.tile_pool(name="tpsum", bufs=1, space="PSUM") as tpsum,
        ):
            # ---- constants / residents ----
            ident = const.tile([P, P], BF)
            make_identity(nc, ident[:])
            ones = const.tile([P, P], BF)
            nc.any.memset(ones[:], 1.0)

            y = resid.tile([P, HC, BL, T], F32, name="y")           # residual, fp32
            xpb = resid.tile([P, HC, BL, T + K - 1], BF, name="xpb")  # conv input, padded
            embT = resid.tile([P, EC, BL, T], BF, name="embT")
            cgb = resid.tile([P, HC, BL, T], BF, name="cgb")
            attnT = resid.tile([P, SC, BL, T], BF, name="attnT")
            comb = resid.tile([P, EC, BL, T], BF, name="comb")
            attT = comb   # alias: attendedT overwrites combined (disjoint live ranges)
            femb = embT   # alias: final emb overwrites embedded (last read is layer L-1)
            encT_sb = resid.tile([P, EC, BL, S], BF, name="encT_sb")
            encC_sb = resid.tile([P, SC, BL, E], BF, name="encC_sb")
            rbc = resid.tile([P, BL, T], F32, name="rbc")           # 1/Z broadcast

            w1_sb = wres.tile([P, HC, E], BF, name="w1_sb")
            w2_sb = wres.tile([P, EC, H], BF, name="w2_sb")
            whid_sb = wres.tile([P, HC, E], BF, name="whid_sb")
            we2h_sb = wres.tile([P, EC, H], BF, name="we2h_sb")
            bias_sb = wres.tile([P, 4 + 2 * L, HC], F32, name="bias_sb")
            # bias_sb columns: 0 y0b, 1 b2s, 2 hidb(EC), 3 spare, 4+2l ba_l, 5+2l bg_l

            nc.sync.dma_start(w1_sb[:], d_w1[:])
            nc.sync.dma_start(w2_sb[:], d_w2s[:])
            nc.sync.dma_start(whid_sb[:], d_whid[:])
            nc.sync.dma_start(we2h_sb[:], d_we2h[:])
            nc.sync.dma_start(encT_sb[:, :, 0, :], d_encT[0])
            nc.sync.dma_start(encT_sb[:, :, 1, :], d_encT[1])
            nc.sync.dma_start(encC_sb[:, :, 0, :], d_encC[0])
            nc.sync.dma_start(encC_sb[:, :, 1, :], d_encC[1])
            if has_y0b:
                nc.sync.dma_start(bias_sb[:, 0, :], d_y0b[:])
            if has_b2s:
                nc.sync.dma_start(bias_sb[:, 1, :], d_b2s[:])
            if has_hidb:
                nc.sync.dma_start(bias_sb[:, 2, :EC], d_hidb[:])
            if has_ba:
                for l in range(L):
                    nc.sync.dma_start(bias_sb[:, 4 + 2 * l, :], d_ba[l])
            if has_bg:
                for l in range(L):
                    nc.sync.dma_start(bias_sb[:, 5 + 2 * l, :], d_bg[l])

            # ---- embedding gather + transpose to [E, T] ----
            for b in range(BL):
                for t4 in range(TC):
                    idx = work.tile([P, 1], I32, tag="idx")
                    nc.sync.dma_start(idx[:], d_tgt[b * T + t4 * P:b * T + (t4 + 1) * P, None])
                    gath = work.tile([P, E], F32, tag="gath")
                    nc.gpsimd.indirect_dma_start(
                        out=gath[:], out_offset=None, in_=d_tok[:, :],
                        in_offset=bass.IndirectOffsetOnAxis(ap=idx[:, :1], axis=0))
                    post = work.tile([P, E], F32, tag="post")
                    nc.sync.dma_start(post[:], d_pos[t4 * P:(t4 + 1) * P, :])
                    epre = work.tile([P, E], BF, tag="epre")
                    nc.vector.tensor_add(out=epre[:], in0=gath[:], in1=post[:])
                    for ec in range(EC):
                        pt = tpsum.tile([P, P], BF, tag="tp")
                        nc.tensor.transpose(pt[:], epre[:, ec * P:(ec + 1) * P], ident[:])
                        nc.scalar.copy(out=embT[:, ec, b, t4 * P:(t4 + 1) * P], in_=pt[:])

            # ---- y0 = 2*emb2hid^T @ embT (+bias) ----
            for b in range(BL):
                for ho in range(HC):
                    ps = cpsum.tile([P, T], F32, tag="cps")
                    for ec in range(EC):
                        nc.tensor.matmul(ps[:], we2h_sb[:, ec, ho * P:(ho + 1) * P],
                                         embT[:, ec, b, :],
                                         start=(ec == 0), stop=(ec == EC - 1))
                    if has_y0b:
                        nc.scalar.activation(y[:, ho, b, :], ps[:], ACT.Identity,
                                             bias=bias_sb[:, 0, ho:ho + 1])
                    else:
                        nc.scalar.copy(out=y[:, ho, b, :], in_=ps[:])
                nc.scalar.copy(out=xpb[:, :, b, K - 1:], in_=y[:, :, b, :])

            # ---- layers ----
            for l in range(L):
                cl = float(c[l])
                for b in range(BL):
                    nc.any.memset(xpb[:, :, b, 0:K - 1], cl)

                # conv + GLU: pairs (a: ho, g: ho+HC)
                for hp in range(HC):
                    wa = wstream.tile([P, HC * K * P], BF, tag="wconv")
                    nc.sync.dma_start(wa[:], d_wc[l, hp])
                    wg = wstream.tile([P, HC * K * P], BF, tag="wconv")
                    nc.sync.dma_start(wg[:], d_wc[l, hp + HC])
                    wa3 = wa.rearrange("p (hi k o) -> p hi k o", hi=HC, k=K)
                    wg3 = wg.rearrange("p (hi k o) -> p hi k o", hi=HC, k=K)
                    for b in range(BL):
                        pa = cpsum.tile([P, T], F32, tag="cps")
                        pg = cpsum.tile([P, T], F32, tag="cps")
                        for hi in range(HC):
                            for k in range(K):
                                st = (hi == 0 and k == 0)
                                sp = (hi == HC - 1 and k == K - 1)
                                nc.tensor.matmul(pa[:], wa3[:, hi, k, :],
                                                 xpb[:, hi, b, k:k + T], start=st, stop=sp)
                        for hi in range(HC):
                            for k in range(K):
                                st = (hi == 0 and k == 0)
                                sp = (hi == HC - 1 and k == K - 1)
                                nc.tensor.matmul(pg[:], wg3[:, hi, k, :],
                                                 xpb[:, hi, b, k:k + T], start=st, stop=sp)
                        th = actbuf.tile([P, T], F32, tag="tanh")
                        if has_bg:
                            nc.scalar.activation(th[:], pg[:], ACT.Tanh, scale=0.5,
                                                 bias=bias_sb[:, 5 + 2 * l, hp:hp + 1])
                        else:
                            nc.scalar.activation(th[:], pg[:], ACT.Tanh, scale=0.5)
                        if has_ba:
                            pa_b = actbuf.tile([P, T], F32, tag="pab")
                            nc.scalar.activation(pa_b[:], pa[:], ACT.Identity,
                                                 bias=bias_sb[:, 4 + 2 * l, hp:hp + 1])
                            a_src = pa_b
                        else:
                            a_src = pa
                        # cg = (tanh + 1) * a   (bf16 out)
                        nc.vector.scalar_tensor_tensor(
                            out=cgb[:, hp, b, :], in0=th[:], scalar=1.0,
                            in1=a_src[:], op0=AluOp.add, op1=AluOp.mult)

                for b in range(BL):
                    # conved_emb -> combined
                    for ec in range(EC):
                        pce = apsum.tile([P, T], F32, tag="aps")
                        for hc_ in range(HC):
                            nc.tensor.matmul(pce[:], w1_sb[:, hc_, ec * P:(ec + 1) * P],
                                             cgb[:, hc_, b, :],
                                             start=(hc_ == 0), stop=(hc_ == HC - 1))
                        nc.vector.scalar_tensor_tensor(
                            out=comb[:, ec, b, :], in0=pce[:], scalar=1.0 / cl,
                            in1=embT[:, ec, b, :], op0=AluOp.mult, op1=AluOp.add)
                    # energyT + exp
                    for sc in range(SC):
                        pen = apsum.tile([P, T], F32, tag="aps")
                        for ec in range(EC):
                            nc.tensor.matmul(pen[:], encT_sb[:, ec, b, sc * P:(sc + 1) * P],
                                             comb[:, ec, b, :],
                                             start=(ec == 0), stop=(ec == EC - 1))
                        nc.scalar.activation(attnT[:, sc, b, :], pen[:], ACT.Exp)
                    # Z broadcast via all-ones matmul, then 1/Z
                    pz = apsum.tile([P, T], F32, tag="aps")
                    for sc in range(SC):
                        nc.tensor.matmul(pz[:], ones[:], attnT[:, sc, b, :],
                                         start=(sc == 0), stop=(sc == SC - 1))
                    nc.vector.reciprocal_approx_fast(out=rbc[:, b, :], in_=pz[:])
                    for sc in range(SC):
                        nc.vector.tensor_mul(out=attnT[:, sc, b, :],
                                             in0=attnT[:, sc, b, :], in1=rbc[:, b, :])
                    # attendedT
                    for ec in range(EC):
                        pat = apsum.tile([P, T], F32, tag="aps")
                        for sc in range(SC):
                            nc.tensor.matmul(pat[:], encC_sb[:, sc, b, ec * P:(ec + 1) * P],
                                             attnT[:, sc, b, :],
                                             start=(sc == 0), stop=(sc == SC - 1))
                        nc.scalar.copy(out=attT[:, ec, b, :], in_=pat[:])
                    # att2 + residual update
                    for ho in range(HC):
                        p2 = apsum.tile([P, T], F32, tag="aps")
                        for ec in range(EC):
                            nc.tensor.matmul(p2[:], w2_sb[:, ec, ho * P:(ho + 1) * P],
                                             attT[:, ec, b, :],
                                             start=(ec == 0), stop=(ec == EC - 1))
                        if has_b2s:
                            p2b = actbuf.tile([P, T], F32, tag="p2b")
                            nc.scalar.activation(p2b[:], p2[:], ACT.Identity,
                                                 bias=bias_sb[:, 1, ho:ho + 1])
                            p2_src = p2b
                        else:
                            p2_src = p2
                        t1 = actbuf.tile([P, T], F32, tag="t1")
                        nc.vector.scalar_tensor_tensor(
                            out=t1[:], in0=p2_src[:], scalar=cl,
                            in1=cgb[:, ho, b, :], op0=AluOp.mult, op1=AluOp.add)
                        nc.vector.tensor_add(out=y[:, ho, b, :], in0=y[:, ho, b, :],
                                             in1=t1[:])
                    # refresh bf16 conv input for next layer / final matmul
                    nc.scalar.copy(out=xpb[:, :, b, K - 1:], in_=y[:, :, b, :])
                    # attention output (last layer): transpose to [T, S] and store
                    if l == L - 1:
                        for t4 in range(TC):
                            ao = outbuf.tile([P, S], F32, tag="ao")
                            for sc in range(SC):
                                pt = tpsum.tile([P, P], BF, tag="tp")
                                nc.tensor.transpose(
                                    pt[:], attnT[:, sc, b, t4 * P:(t4 + 1) * P], ident[:])
                                nc.vector.tensor_copy(out=ao[:, sc * P:(sc + 1) * P], in_=pt[:])
                            nc.sync.dma_start(d_attn[b, t4 * P:(t4 + 1) * P, :], ao[:])

            # ---- final projection: femb = whid^T @ y_L (+hidb) ----
            for b in range(BL):
                for ec in range(EC):
                    pf = cpsum.tile([P, T], F32, tag="cps")
                    for hc_ in range(HC):
                        nc.tensor.matmul(pf[:], whid_sb[:, hc_, ec * P:(ec + 1) * P],
                                         xpb[:, hc_, b, K - 1:],
                                         start=(hc_ == 0), stop=(hc_ == HC - 1))
                    if has_hidb:
                        nc.scalar.activation(femb[:, ec, b, :], pf[:], ACT.Identity,
                                             bias=bias_sb[:, 2, ec:ec + 1])
                    else:
                        nc.scalar.copy(out=femb[:, ec, b, :], in_=pf[:])

            # ---- fc_out: logits[t, v] ----
            for vt in range(VT):
                wf = wstream.tile([P, EC * 512], BF, tag="wfc")
                nc.sync.dma_start(wf[:], d_wfc[:, vt, :])
                wf3 = wf.rearrange("p (e n) -> p e n", e=EC)
                vw = min(512, V - vt * 512)
                for b in range(BL):
                    for t4 in range(TC):
                        pl = cpsum.tile([P, 512], F32, tag="cps")
                        for ec in range(EC):
                            nc.tensor.matmul(pl[:], femb[:, ec, b, t4 * P:(t4 + 1) * P],
                                             wf3[:, ec, :],
                                             start=(ec == 0), stop=(ec == EC - 1))
                        ot = outbuf.tile([P, 512], F32, tag="ot")
                        if (b * TC + t4) % 2 == 0:
                            nc.scalar.copy(out=ot[:, :vw], in_=pl[:, :vw])
                        else:
                            nc.vector.tensor_copy(out=ot[:, :vw], in_=pl[:, :vw])
                        nc.sync.dma_start(
                            d_out[b, t4 * P:(t4 + 1) * P, vt * 512:vt * 512 + vw],
                            ot[:, :vw])
    return nc


_CACHE = {}


def _run(inputs, trace=False):
    pp = _prepack(inputs)
    if "nc" not in _CACHE:
        nc = _build_program(pp)
        nc.compile()
        nc.m = get_hw_module(nc.m)
        _CACHE["nc"] = nc
    nc = _CACHE["nc"]

    shared_keys = ["tok2", "pos2", "we2h", "wc", "w1", "w2s", "whid", "wfc"]
    in_maps = []
    for core in range(NCORES):
        m = {k: pp[k] for k in shared_keys}
        bsl = slice(core * BL, (core + 1) * BL)
        m["tgt32"] = np.ascontiguousarray(pp["tgt32"][bsl].reshape(-1))
        m["encT"] = np.ascontiguousarray(pp["encT"][bsl])
        m["encC"] = np.ascontiguousarray(pp["encC"][bsl])
        if np.any(pp["y0b"]):
            m["y0b"] = pp["y0b"]
        if np.any(pp["ba"]):
            m["ba"] = pp["ba"]
        if np.any(pp["bg"]):
            m["bg"] = pp["bg"]
        if np.any(pp["b2s"]):
            m["b2s"] = pp["b2s"]
        if np.any(pp["hidb"]):
            m["hidb"] = pp["hidb"]
        in_maps.append(m)

    res = run_bass_kernel_spmd(nc, in_maps, core_ids=list(range(NCORES)),
                               trace=trace)
    out = np.concatenate([r["logits"] for r in res.results], axis=0)
    attn = np.concatenate([r["attno"] for r in res.results], axis=0)
    if np.any(pp["fcb"]):
        out = out + pp["fcb"][None, None, :]
    return (out, attn), res


def kernel(**inputs):
    (out, attn), _ = _run(inputs, trace=False)
    return out, attn


# revision 9
# speedup vs baseline: 1.0740x; 1.0312x over previous
"""Trainium2 Bass kernel for nn_Decoder (conv-seq2seq decoder).

Strategy: data-parallel over batch, 2 batch elements per NeuronCore (8 cores).
Each core runs the full 6-layer decoder on its 2 batch elements in bf16
matmuls with fp32 accumulation. Weights are host-prepacked into SBUF-friendly
layouts; the residual stream is carried in a per-layer scaled representation
y_l = sqrt(2)^(l+1) * x_l so every layer update is a pure add (all scale
factors are folded into weights or single fused DVE ops).

Self-contained: hardcodes shapes B=16,T=512,S=512,V=10000,E=512,H=1024,L=6,K=3.
"""
import os
import sys
import numpy as np
import ml_dtypes

for _p in ("/opt/trn_rl_repo", "/root/.axon_site/_ro/trn_rl_repo"):
    if os.path.isdir(_p) and _p not in sys.path:
        sys.path.insert(0, _p)

import concourse.bass as bass
import concourse.bacc as bacc
import concourse.mybir as mybir
import concourse.tile as tile
from concourse.bass_utils import run_bass_kernel_spmd
from concourse.bass_interp import get_hw_module
from concourse.masks import make_identity

bf16 = ml_dtypes.bfloat16
f32 = np.float32
BF = mybir.dt.bfloat16
F32 = mybir.dt.float32
I32 = mybir.dt.int32

B, T, S = 16, 512, 512
V, E, H = 10000, 512, 1024
L, K = 6, 3
NCORES = 8
BL = B // NCORES          # batch per core
P = 128
EC, SC, TC, HC, OC = E // P, S // P, T // P, H // P, 2 * H // P   # 4,4,4,8,16
VT = 20                   # v tiles of 512 (V padded 10000 -> 10240)
AluOp = mybir.AluOpType
ACT = mybir.ActivationFunctionType


def _prepack(inputs):
    """Host-side fold + layout prep. Returns dict of per-core-shared arrays
    (weights) and the per-batch arrays to shard."""
    s = f32(np.sqrt(0.5))
    rt2 = f32(np.sqrt(2.0))
    c = np.array([rt2 * rt2 ** l for l in range(L + 1)], f32)

    g = {k: np.asarray(v) for k, v in inputs.items()}
    conv_w = g["conv_w"].astype(f32)
    b1 = g["attn_hid2emb_b"].astype(f32)

    out = {}
    out["tok2"] = (s * g["tok_emb"].astype(f32)).astype(f32)                 # [V, E]
    out["pos2"] = (s * (g["pos_emb"][:T].astype(f32) + b1[None, :])).astype(f32)  # [T, E]
    out["we2h"] = np.ascontiguousarray(
        (2.0 * g["emb2hid_w"].astype(f32)).astype(bf16)
        .reshape(EC, P, H).transpose(1, 0, 2))                                # [P, EC, H]
    out["y0b"] = (rt2 * (g["emb2hid_b"].astype(f32) - b1 @ g["emb2hid_w"].astype(f32))
                  ).astype(f32).reshape(HC, P).T.copy()                       # [P, HC]
    # conv weights: a-half scaled s/2 (GLU computes psum*(tanh+1)), g-half /c_l
    wdev = np.empty((L, 2 * H, H, K), f32)
    wdev[:, :H] = (s / 2.0) * conv_w[:, :H]
    wdev[:, H:] = conv_w[:, H:] / c[:L, None, None, None]
    # [L, OC, P(i_lo), HC(hi), K, P(o_lo)] flattened to [L, OC, P, HC*K*P]
    wc = wdev.reshape(L, OC, P, HC, P, K).transpose(0, 1, 4, 3, 5, 2)
    out["wc"] = np.ascontiguousarray(wc.astype(bf16)).reshape(L, OC, P, HC * K * P)
    out["ba"] = ((c[:L, None] * s / 2.0) * g["conv_b"][:, :H].astype(f32)
                 ).reshape(L, HC, P).transpose(0, 2, 1).copy()                # [L, P, HC]
    out["bg"] = (0.5 * g["conv_b"][:, H:].astype(f32)
                 ).reshape(L, HC, P).transpose(0, 2, 1).copy()                # [L, P, HC]
    out["w1"] = np.ascontiguousarray(
        g["attn_hid2emb_w"].astype(f32).astype(bf16).reshape(HC, P, E).transpose(1, 0, 2))
    out["w2s"] = np.ascontiguousarray(
        (s * g["attn_emb2hid_w"].astype(f32)).astype(bf16).reshape(EC, P, H).transpose(1, 0, 2))
    out["b2s"] = (s * g["attn_emb2hid_b"].astype(f32)).reshape(HC, P).T.copy()  # [P, HC]
    out["whid"] = np.ascontiguousarray(
        (g["hid2emb_w"].astype(f32) / c[L]).astype(bf16).reshape(HC, P, E).transpose(1, 0, 2))
    out["hidb"] = g["hid2emb_b"].astype(f32).reshape(EC, P).T.copy()          # [P, EC]
    wfc = np.zeros((E, VT * 512), f32)
    wfc[:, :V] = g["fc_out_w"].astype(f32)
    out["wfc"] = np.ascontiguousarray(
        wfc.astype(bf16).reshape(EC, P, VT, 512).transpose(1, 2, 0, 3)
    ).reshape(P, VT, EC * 512)                                                # [P, VT, EC*512]
    out["fcb"] = g["fc_out_b"].astype(f32)                                    # host-applied if nonzero
    # per-batch tensors
    out["tgt32"] = g["tgt"].astype(np.int32)                                  # [B, T]
    out["encT"] = np.ascontiguousarray(
        g["encoder_conved"].astype(f32).astype(bf16)
        .reshape(B, S, EC, P).transpose(0, 3, 2, 1))                          # [B, P, EC, S]
    out["encC"] = np.ascontiguousarray(
        g["encoder_combined"].astype(f32).astype(bf16)
        .reshape(B, SC, P, E).transpose(0, 2, 1, 3))                          # [B, P, SC, E]
    out["c"] = c
    return out


def _build_program(pp):
    """Emit the per-core Bass/Tile program. Returns nc."""
    s = f32(np.sqrt(0.5))
    c = pp["c"]
    has_ba = bool(np.any(pp["ba"]))
    has_bg = bool(np.any(pp["bg"]))
    has_y0b = bool(np.any(pp["y0b"]))
    has_b2s = bool(np.any(pp["b2s"]))
    has_hidb = bool(np.any(pp["hidb"]))

    nc = bacc.Bacc("TRN2", target_bir_lowering=False, debug=False,
                   num_devices=NCORES)

    # ---- DRAM I/O ----
    d_tgt = nc.dram_tensor("tgt32", [BL * T], I32, kind="ExternalInput")
    d_tok = nc.dram_tensor("tok2", [V, E], F32, kind="ExternalInput")
    d_pos = nc.dram_tensor("pos2", [T, E], F32, kind="ExternalInput")
    d_we2h = nc.dram_tensor("we2h", [P, EC, H], BF, kind="ExternalInput")
    d_wc = nc.dram_tensor("wc", [L, OC, P, HC * K * P], BF, kind="ExternalInput")
    d_w1 = nc.dram_tensor("w1", [P, HC, E], BF, kind="ExternalInput")
    d_w2s = nc.dram_tensor("w2s", [P, EC, H], BF, kind="ExternalInput")
    d_whid = nc.dram_tensor("whid", [P, HC, E], BF, kind="ExternalInput")
    d_wfc = nc.dram_tensor("wfc", [P, VT, EC * 512], BF, kind="ExternalInput")
    d_encT = nc.dram_tensor("encT", [BL, P, EC, S], BF, kind="ExternalInput")
    d_encC = nc.dram_tensor("encC", [BL, P, SC, E], BF, kind="ExternalInput")
    d_y0b = nc.dram_tensor("y0b", [P, HC], F32, kind="ExternalInput") if has_y0b else None
    d_ba = nc.dram_tensor("ba", [L, P, HC], F32, kind="ExternalInput") if has_ba else None
    d_bg = nc.dram_tensor("bg", [L, P, HC], F32, kind="ExternalInput") if has_bg else None
    d_b2s = nc.dram_tensor("b2s", [P, HC], F32, kind="ExternalInput") if has_b2s else None
    d_hidb = nc.dram_tensor("hidb", [P, EC], F32, kind="ExternalInput") if has_hidb else None
    d_out = nc.dram_tensor("logits", [BL, T, V], F32, kind="ExternalOutput")
    d_attn = nc.dram_tensor("attno", [BL, T, S], F32, kind="ExternalOutput")

    with tile.TileContext(nc) as tc:
        with (
            tc.tile_pool(name="const", bufs=1) as const,
            tc.tile_pool(name="resid", bufs=1) as resid,
            tc.tile_pool(name="wres", bufs=1) as wres,
            tc.tile_pool(name="wstream", bufs=3) as wstream,
            tc.tile_pool(name="work", bufs=2) as work,
            tc.tile_pool(name="actbuf", bufs=2) as actbuf,
            tc.tile_pool(name="outbuf", bufs=4) as outbuf,
            tc.tile_pool(name="cpsum", bufs=4, space="PSUM") as cpsum,
            tc.tile_pool(name="apsum", bufs=3, space="PSUM") as apsum,
            tc.tile_pool(name="tpsum", bufs=1, space="PSUM") as tpsum,
        ):
            # ---- constants / residents ----
            ident = const.tile([P, P], BF)
            make_identity(nc, ident[:])
            ones = const.tile([P, P], BF)
            nc.any.memset(ones[:], 1.0)

            y = resid.tile([P, HC, BL, T], F32, name="y")           # residual, fp32
            xpb = resid.tile([P, HC, BL, T + K - 1], BF, name="xpb")  # conv input, padded
            embT = resid.tile([P, EC, BL, T], BF, name="embT")
            cgb = resid.tile([P, HC, BL, T], BF, name="cgb")
            attnT = resid.tile([P, SC, BL, T], BF, name="attnT")
            comb = resid.tile([P, EC, BL, T], BF, name="comb")
            attT = comb   # alias: attendedT overwrites combined (disjoint live ranges)
            femb = embT   # alias: final emb overwrites embedded (last read is layer L-1)
            encT_sb = resid.tile([P, EC, BL, S], BF, name="encT_sb")
            encC_sb = resid.tile([P, SC, BL, E], BF, name="encC_sb")
            rbc = resid.tile([P, BL, T], F32, name="rbc")           # 1/Z broadcast

            w1_sb = wres.tile([P, HC, E], BF, name="w1_sb")
            w2_sb = wres.tile([P, EC, H], BF, name="w2_sb")
            whid_sb = wres.tile([P, HC, E], BF, name="whid_sb")
            we2h_sb = wres.tile([P, EC, H], BF, name="we2h_sb")
            bias_sb = wres.tile([P, 4 + 2 * L, HC], F32, name="bias_sb")
            # bias_sb columns: 0 y0b, 1 b2s, 2 hidb(EC), 3 spare, 4+2l ba_l, 5+2l bg_l

            for ec in range(EC):
                nc.sync.dma_start(we2h_sb[:, ec, :], d_we2h[:, ec, :])
            nc.sync.dma_start(w1_sb[:], d_w1[:])
            nc.sync.dma_start(w2_sb[:], d_w2s[:])
            nc.sync.dma_start(whid_sb[:], d_whid[:])
            nc.sync.dma_start(encT_sb[:, :, 0, :], d_encT[0])
            nc.sync.dma_start(encT_sb[:, :, 1, :], d_encT[1])
            nc.sync.dma_start(encC_sb[:, :, 0, :], d_encC[0])
            nc.sync.dma_start(encC_sb[:, :, 1, :], d_encC[1])
            if has_y0b:
                nc.sync.dma_start(bias_sb[:, 0, :], d_y0b[:])
            if has_b2s:
                nc.sync.dma_start(bias_sb[:, 1, :], d_b2s[:])
            if has_hidb:
                nc.sync.dma_start(bias_sb[:, 2, :EC], d_hidb[:])
            if has_ba:
                for l in range(L):
                    nc.sync.dma_start(bias_sb[:, 4 + 2 * l, :], d_ba[l])
            if has_bg:
                for l in range(L):
                    nc.sync.dma_start(bias_sb[:, 5 + 2 * l, :], d_bg[l])

            # ---- embedding gather + transpose to [E, T] ----
            for b in range(BL):
                for t4 in range(TC):
                    idx = work.tile([P, 1], I32, tag="idx")
                    nc.sync.dma_start(idx[:], d_tgt[b * T + t4 * P:b * T + (t4 + 1) * P, None])
                    gath = work.tile([P, E], F32, tag="gath")
                    nc.gpsimd.indirect_dma_start(
                        out=gath[:], out_offset=None, in_=d_tok[:, :],
                        in_offset=bass.IndirectOffsetOnAxis(ap=idx[:, :1], axis=0))
                    post = work.tile([P, E], F32, tag="post")
                    nc.sync.dma_start(post[:], d_pos[t4 * P:(t4 + 1) * P, :])
                    epre = work.tile([P, E], BF, tag="epre")
                    nc.vector.tensor_add(out=epre[:], in0=gath[:], in1=post[:])
                    for ec in range(EC):
                        pt = tpsum.tile([P, P], BF, tag="tp")
                        nc.tensor.transpose(pt[:], epre[:, ec * P:(ec + 1) * P], ident[:])
                        nc.scalar.copy(out=embT[:, ec, b, t4 * P:(t4 + 1) * P], in_=pt[:])

            # ---- y0 = 2*emb2hid^T @ embT (+bias) ----
            for b in range(BL):
                for ho in range(HC):
                    ps = cpsum.tile([P, T], F32, tag="cps")
                    for ec in range(EC):
                        nc.tensor.matmul(ps[:], we2h_sb[:, ec, ho * P:(ho + 1) * P],
                                         embT[:, ec, b, :],
                                         start=(ec == 0), stop=(ec == EC - 1))
                    if has_y0b:
                        nc.scalar.activation(y[:, ho, b, :], ps[:], ACT.Identity,
                                             bias=bias_sb[:, 0, ho:ho + 1])
                    else:
                        nc.scalar.copy(out=y[:, ho, b, :], in_=ps[:])
                nc.scalar.copy(out=xpb[:, :, b, K - 1:], in_=y[:, :, b, :])

            # ---- layers ----
            for l in range(L):
                cl = float(c[l])
                for b in range(BL):
                    nc.any.memset(xpb[:, :, b, 0:K - 1], cl)

                # conv + GLU: pairs (a: ho, g: ho+HC)
                for hp in range(HC):
                    wa = wstream.tile([P, HC * K * P], BF, tag="wconv")
                    nc.sync.dma_start(wa[:], d_wc[l, hp])
                    wg = wstream.tile([P, HC * K * P], BF, tag="wconv")
                    nc.sync.dma_start(wg[:], d_wc[l, hp + HC])
                    wa3 = wa.rearrange("p (hi k o) -> p hi k o", hi=HC, k=K)
                    wg3 = wg.rearrange("p (hi k o) -> p hi k o", hi=HC, k=K)
                    for b in range(BL):
                        pa = cpsum.tile([P, T], F32, tag="cps")
                        pg = cpsum.tile([P, T], F32, tag="cps")
                        for hi in range(HC):
                            for k in range(K):
                                st = (hi == 0 and k == 0)
                                sp = (hi == HC - 1 and k == K - 1)
                                nc.tensor.matmul(pa[:], wa3[:, hi, k, :],
                                                 xpb[:, hi, b, k:k + T], start=st, stop=sp)
                        for hi in range(HC):
                            for k in range(K):
                                st = (hi == 0 and k == 0)
                                sp = (hi == HC - 1 and k == K - 1)
                                nc.tensor.matmul(pg[:], wg3[:, hi, k, :],
                                                 xpb[:, hi, b, k:k + T], start=st, stop=sp)
                        th = actbuf.tile([P, T], F32, tag="tanh")
                        if has_bg:
                            nc.scalar.activation(th[:], pg[:], ACT.Tanh, scale=0.5,
                                                 bias=bias_sb[:, 5 + 2 * l, hp:hp + 1])
                        else:
                            nc.scalar.activation(th[:], pg[:], ACT.Tanh, scale=0.5)
                        if has_ba:
                            pa_b = actbuf.tile([P, T], F32, tag="pab")
                            nc.scalar.activation(pa_b[:], pa[:], ACT.Identity,
                                                 bias=bias_sb[:, 4 + 2 * l, hp:hp + 1])
                            a_src = pa_b
                        else:
                            a_src = pa
                        # cg = (tanh + 1) * a   (bf16 out)
                        nc.vector.scalar_tensor_tensor(
                            out=cgb[:, hp, b, :], in0=th[:], scalar=1.0,
                            in1=a_src[:], op0=AluOp.add, op1=AluOp.mult)

                for b in range(BL):
                    # conved_emb -> combined
                    for ec in range(EC):
                        pce = apsum.tile([P, T], F32, tag="aps")
                        for hc_ in range(HC):
                            nc.tensor.matmul(pce[:], w1_sb[:, hc_, ec * P:(ec + 1) * P],
                                             cgb[:, hc_, b, :],
                                             start=(hc_ == 0), stop=(hc_ == HC - 1))
                        nc.vector.scalar_tensor_tensor(
                            out=comb[:, ec, b, :], in0=pce[:], scalar=1.0 / cl,
                            in1=embT[:, ec, b, :], op0=AluOp.mult, op1=AluOp.add)
                    # energyT + exp
                    for sc in range(SC):
                        pen = apsum.tile([P, T], F32, tag="aps")
                        for ec in range(EC):
                            nc.tensor.matmul(pen[:], encT_sb[:, ec, b, sc * P:(sc + 1) * P],
                                             comb[:, ec, b, :],
                                             start=(ec == 0), stop=(ec == EC - 1))
                        nc.scalar.activation(attnT[:, sc, b, :], pen[:], ACT.Exp)
                    # Z broadcast via all-ones matmul, then 1/Z
                    pz = apsum.tile([P, T], F32, tag="aps")
                    for sc in range(SC):
                        nc.tensor.matmul(pz[:], ones[:], attnT[:, sc, b, :],
                                         start=(sc == 0), stop=(sc == SC - 1))
                    nc.vector.reciprocal_approx_fast(out=rbc[:, b, :], in_=pz[:])
                    for sc in range(SC):
                        nc.vector.tensor_mul(out=attnT[:, sc, b, :],
                                             in0=attnT[:, sc, b, :], in1=rbc[:, b, :])
                    # attendedT
                    for ec in range(EC):
                        pat = apsum.tile([P, T], F32, tag="aps")
                        for sc in range(SC):
                            nc.tensor.matmul(pat[:], encC_sb[:, sc, b, ec * P:(ec + 1) * P],
                                             attnT[:, sc, b, :],
                                             start=(sc == 0), stop=(sc == SC - 1))
                        nc.scalar.copy(out=attT[:, ec, b, :], in_=pat[:])
                    # att2 + residual update
                    for ho in range(HC):
                        p2 = apsum.tile([P, T], F32, tag="aps")
                        for ec in range(EC):
                            nc.tensor.matmul(p2[:], w2_sb[:, ec, ho * P:(ho + 1) * P],
                                             attT[:, ec, b, :],
                                             start=(ec == 0), stop=(ec == EC - 1))
                        if has_b2s:
                            p2b = actbuf.tile([P, T], F32, tag="p2b")
                            nc.scalar.activation(p2b[:], p2[:], ACT.Identity,
                                                 bias=bias_sb[:, 1, ho:ho + 1])
                            p2_src = p2b
                        else:
                            p2_src = p2
                        t1 = actbuf.tile([P, T], F32, tag="t1")
                        nc.vector.scalar_tensor_tensor(
                            out=t1[:], in0=p2_src[:], scalar=cl,
                            in1=cgb[:, ho, b, :], op0=AluOp.mult, op1=AluOp.add)
                        nc.vector.tensor_add(out=y[:, ho, b, :], in0=y[:, ho, b, :],
                                             in1=t1[:])
                        # refresh bf16 conv input for next layer / final matmul
                        nc.scalar.copy(out=xpb[:, ho, b, K - 1:], in_=y[:, ho, b, :])
                    # attention output (last layer): transpose to [T, S] and store
                    if l == L - 1:
                        for t4 in range(TC):
                            ao = outbuf.tile([P, S], F32, tag="ao")
                            for sc in range(SC):
                                pt = tpsum.tile([P, P], BF, tag="tp")
                                nc.tensor.transpose(
                                    pt[:], attnT[:, sc, b, t4 * P:(t4 + 1) * P], ident[:])
                                nc.vector.tensor_copy(out=ao[:, sc * P:(sc + 1) * P], in_=pt[:])
                            nc.sync.dma_start(d_attn[b, t4 * P:(t4 + 1) * P, :], ao[:])

            # ---- final projection: femb = whid^T @ y_L (+hidb) ----
            for b in range(BL):
                for ec in range(EC):
                    pf = cpsum.tile([P, T], F32, tag="cps")
                    for hc_ in range(HC):
                        nc.tensor.matmul(pf[:], whid_sb[:, hc_, ec * P:(ec + 1) * P],
                                         xpb[:, hc_, b, K - 1:],
                                         start=(hc_ == 0), stop=(hc_ == HC - 1))
                    if has_hidb:
                        nc.scalar.activation(femb[:, ec, b, :], pf[:], ACT.Identity,
                                             bias=bias_sb[:, 2, ec:ec + 1])
                    else:
                        nc.scalar.copy(out=femb[:, ec, b, :], in_=pf[:])

            # ---- fc_out: logits[t, v] ----
            for vt in range(VT):
                wf = wstream.tile([P, EC * 512], BF, tag="wfc")
                nc.sync.dma_start(wf[:], d_wfc[:, vt, :])
                wf3 = wf.rearrange("p (e n) -> p e n", e=EC)
                vw = min(512, V - vt * 512)
                for b in range(BL):
                    for t4 in range(TC):
                        pl = cpsum.tile([P, 512], F32, tag="cps")
                        for ec in range(EC):
                            nc.tensor.matmul(pl[:], femb[:, ec, b, t4 * P:(t4 + 1) * P],
                                             wf3[:, ec, :],
                                             start=(ec == 0), stop=(ec == EC - 1))
                        ot = outbuf.tile([P, 512], F32, tag="ot")
                        if (b * TC + t4) % 2 == 0:
                            nc.scalar.copy(out=ot[:, :vw], in_=pl[:, :vw])
                        else:
                            nc.vector.tensor_copy(out=ot[:, :vw], in_=pl[:, :vw])
                        nc.sync.dma_start(
                            d_out[b, t4 * P:(t4 + 1) * P, vt * 512:vt * 512 + vw],
                            ot[:, :vw])
    return nc


_CACHE = {}


def _run(inputs, trace=False):
    pp = _prepack(inputs)
    if "nc" not in _CACHE:
        nc = _build_program(pp)
        nc.compile()
        nc.m = get_hw_module(nc.m)
        _CACHE["nc"] = nc
    nc = _CACHE["nc"]

    shared_keys = ["tok2", "pos2", "we2h", "wc", "w1", "w2s", "whid", "wfc"]
    in_maps = []
    for core in range(NCORES):
        m = {k: pp[k] for k in shared_keys}
        bsl = slice(core * BL, (core + 1) * BL)
        m["tgt32"] = np.ascontiguousarray(pp["tgt32"][bsl].reshape(-1))
        m["encT"] = np.ascontiguousarray(pp["encT"][bsl])
        m["encC"] = np.ascontiguousarray(pp["encC"][bsl])
        if np.any(pp["y0b"]):
            m["y0b"] = pp["y0b"]
        if np.any(pp["ba"]):
            m["ba"] = pp["ba"]
        if np.any(pp["bg"]):
            m["bg"] = pp["bg"]
        if np.any(pp["b2s"]):
            m["b2s"] = pp["b2s"]
        if np.any(pp["hidb"]):
            m["hidb"] = pp["hidb"]
        in_maps.append(m)

    res = run_bass_kernel_spmd(nc, in_maps, core_ids=list(range(NCORES)),
                               trace=trace)
    out = np.concatenate([r["logits"] for r in res.results], axis=0)
    attn = np.concatenate([r["attno"] for r in res.results], axis=0)
    if np.any(pp["fcb"]):
        out = out + pp["fcb"][None, None, :]
    return (out, attn), res


def kernel(**inputs):
    (out, attn), _ = _run(inputs, trace=False)
    return out, attn
